# revision 3
# baseline (speedup 1.0000x reference)
"""Trainium2 Bass kernel for nn_MultiHeadDotProductAttention_76725295776285.

Full multi-head attention (B=2, Q=K=4096, F=512, H=8, D=64) on 8 NeuronCores.

Sharding: core c handles batch b = c//4 and q-rows [(c%4)*1024, (c%4+1)*1024).
Each core computes all 8 heads for its q-slice (K/V projection for its batch is
recomputed on each of the 4 cores sharing that batch), so the output projection
sums over heads locally and no collective is needed.

Device-side dataflow (per core):
  - activations are fed pre-transposed ([F, seq]) so every matmul has its
    contraction dim on partitions with no on-chip transposes
  - Q^T [hd, q], K^T [hd, k] kept in float32r (TF32-like, full PE rate)
  - V evacuated to bf16 as [k, head, 65] with a ones column, so the AV matmul
    produces the softmax denominator in its 65th output row for free
  - S^T = K_h Q_h^T per head via two row-packed (tile_position) Kc=64 matmuls
  - softmax without max-subtraction (logits std ~8, |logit| < ~60 is fp32-safe)
  - exp on the Scalar engine PSUM->SBUF in [128, 1024] slabs, bf16 out
  - out^T accumulated in PSUM over all 32 k-chunks, normalized by the
    reciprocal denominator broadcast across partitions with a tiny matmul
  - output projection in bf16, accumulate over hd chunks, DMA out fp32
"""

import os
import sys

for _p in ("/opt/trn_rl_repo", "/root/.axon_site/_ro/trn_rl_repo"):
    if os.path.isdir(_p) and _p not in sys.path:
        sys.path.append(_p)

import numpy as np

import concourse.bacc as bacc
import concourse.tile as tile
from concourse import mybir
from concourse.bass_utils import run_bass_kernel_spmd

B, Q, K, F, H, D = 2, 4096, 4096, 512, 8, 64
HD = H * D            # 512
NCORES = 8
QSH = Q // 4          # 1024 q rows per core
FC = F // 128         # 4 F chunks
HDC = HD // 128       # 4 hd chunks
NKB = K // 512        # 8 k blocks (DMA/projection granularity)
NKC = K // 128        # 32 k chunks (attention granularity)
NQB = QSH // 512      # 2 q blocks per core
NHP = H // 2          # 4 head pairs

f32 = mybir.dt.float32
f32r = mybir.dt.float32r
bf16 = mybir.dt.bfloat16

_cache = {}
last_result = None  # BassKernelResults of the most recent run (for profiling)


def _build_program():
    nc = bacc.Bacc("TRN2", target_bir_lowering=False, debug=False,
                   num_devices=NCORES)

    xqT = nc.dram_tensor("xqT", [F, QSH], f32r, kind="ExternalInput")
    xkvT = nc.dram_tensor("xkvT", [F, K], f32r, kind="ExternalInput")
    wq = nc.dram_tensor("wq", [F, HD], f32r, kind="ExternalInput")
    wk = nc.dram_tensor("wk", [F, HD], f32r, kind="ExternalInput")
    wv = nc.dram_tensor("wv", [F, HD], f32r, kind="ExternalInput")
    wo = nc.dram_tensor("wo", [HD, F], f32, kind="ExternalInput")
    ones64 = nc.dram_tensor("ones64", [1, 64], f32r, kind="ExternalInput")
    out = nc.dram_tensor("out", [QSH, F], f32, kind="ExternalOutput")

    # partition-major views: row index (c*128 + p) -> [p, c, :]
    xqT_r = xqT.rearrange("(c p) q -> p c q", p=128)
    xkvT_r = xkvT.rearrange("(c p) k -> p c k", p=128)
    wq_r = wq.rearrange("(c p) n -> p c n", p=128)
    wk_r = wk.rearrange("(c p) n -> p c n", p=128)
    wv_r = wv.rearrange("(c p) n -> p c n", p=128)
    wo_r = wo.rearrange("(c p) n -> p c n", p=128)

    with tile.TileContext(nc) as tc:
        with (
            tc.tile_pool(name="persist", bufs=1) as persist,
            tc.tile_pool(name="stream", bufs=2) as stream,
            tc.tile_pool(name="ptp", bufs=4) as ptp,
            tc.tile_pool(name="small", bufs=4) as small,
            tc.tile_pool(name="psum", bufs=2, space="PSUM") as psum,
        ):
            # ---- persistent SBUF tensors ----
            qT_sb = persist.tile([128, HDC, QSH], f32r, tag="qT")
            kT_sb = [persist.tile([128, HDC, 512], f32r, tag=f"kT{kb}",
                                  name=f"kT{kb}")
                     for kb in range(NKB)]
            v_sb = [persist.tile([128, H, 65], bf16, tag=f"v{kc}",
                                 name=f"v{kc}")
                    for kc in range(NKC)]
            outT_sb = persist.tile([128, HDC, QSH], bf16, tag="outT")
            wk_sb = persist.tile([128, FC, HD], f32r, tag="wk")
            wv_sb = persist.tile([128, FC, HD], f32r, tag="wv")
            wq_sb = persist.tile([128, FC, HD], f32r, tag="wq")
            wo_bf = persist.tile([128, HDC, F], bf16, tag="wo_bf")
            ones_sb = persist.tile([1, 64], f32r, tag="ones")

            # ---- input DMAs (HWDGE via sync engine) ----
            nc.sync.dma_start(out=wq_sb[:], in_=wq_r[:])
            nc.sync.dma_start(out=wk_sb[:], in_=wk_r[:])
            nc.sync.dma_start(out=wv_sb[:], in_=wv_r[:])
            nc.sync.dma_start(out=ones_sb[:], in_=ones64[:])
            wo_f32 = persist.tile([128, HDC, F], f32, tag="wo_f32")
            nc.sync.dma_start(out=wo_f32[:], in_=wo_r[:])
            nc.vector.tensor_copy(wo_bf[:], wo_f32[:])

            # ---- phase 1a: Q^T projection ----
            # Q^T[hd, q] = sum_F wq[F, hd]^T x xqT[F, q]
            for qb in range(NQB):
                xq_blk = stream.tile([128, FC, 512], f32r, tag="xblk")
                nc.sync.dma_start(out=xq_blk[:],
                                  in_=xqT_r[:, :, qb * 512:(qb + 1) * 512])
                for hdc in range(HDC):
                    acc = psum.tile([128, 512], f32, tag="proj")
                    for fc in range(FC):
                        nc.tensor.matmul(
                            acc[:],
                            wq_sb[:, fc, hdc * 128:(hdc + 1) * 128],
                            xq_blk[:, fc, :],
                            start=(fc == 0), stop=(fc == FC - 1),
                        )
                    nc.vector.tensor_copy(
                        qT_sb[:, hdc, qb * 512:(qb + 1) * 512], acc[:])

            # ---- phase 1b: K^T and V projections, per k block ----
            for kb in range(NKB):
                xkv_blk = stream.tile([128, FC, 512], f32r, tag="xblk")
                nc.sync.dma_start(out=xkv_blk[:],
                                  in_=xkvT_r[:, :, kb * 512:(kb + 1) * 512])
                # K^T[hd, kblock]
                for hdc in range(HDC):
                    acc = psum.tile([128, 512], f32, tag="proj")
                    for fc in range(FC):
                        nc.tensor.matmul(
                            acc[:],
                            wk_sb[:, fc, hdc * 128:(hdc + 1) * 128],
                            xkv_blk[:, fc, :],
                            start=(fc == 0), stop=(fc == FC - 1),
                        )
                    nc.vector.tensor_copy(kT_sb[kb][:, hdc, :], acc[:])
                # V[kblock, hd] (natural layout, k on partitions)
                for ks in range(4):
                    kc = kb * 4 + ks
                    acc = psum.tile([128, 512], f32, tag="proj")
                    for fc in range(FC):
                        nc.tensor.matmul(
                            acc[:],
                            xkv_blk[:, fc, ks * 128:(ks + 1) * 128],
                            wv_sb[:, fc, :],
                            start=(fc == 0), stop=(fc == FC - 1),
                        )
                    nc.vector.tensor_copy(
                        v_sb[kc][:, :, 0:64],
                        acc.rearrange("p (h d) -> p h d", h=H))
                    nc.vector.memset(v_sb[kc][:, :, 64:65], 1.0)

            # ---- phase 2: attention, per head pair / q block ----
            for hp in range(NHP):
                for qb in range(NQB):
                    av = [psum.tile([65, 512], f32, tag="av", name=f"av{hp}_{qb}_{hi}")
                          for hi in range(2)]
                    for kc in range(NKC):
                        kb, ks = kc // 4, kc % 4
                        st = psum.tile([128, 2, 512], f32, tag="st",
                                       name=f"st{hp}_{qb}_{kc}")
                        for hi in range(2):
                            nc.tensor.matmul(
                                st[:, hi, :],
                                kT_sb[kb][hi * 64:(hi + 1) * 64, hp,
                                          ks * 128:(ks + 1) * 128],
                                qT_sb[hi * 64:(hi + 1) * 64, hp,
                                      qb * 512:(qb + 1) * 512],
                                start=True, stop=True,
                                tile_position=(hi * 64, 0),
                            )
                        pT = ptp.tile([128, 2, 512], bf16, tag="pT",
                                      name=f"pT{hp}_{qb}_{kc}")
                        nc.scalar.activation(
                            out=pT[:], in_=st[:],
                            func=mybir.ActivationFunctionType.Exp)
                        for hi in range(2):
                            nc.tensor.matmul(
                                av[hi][:],
                                v_sb[kc][:, hp * 2 + hi, :],
                                pT[:, hi, :],
                                start=(kc == 0), stop=(kc == NKC - 1),
                            )
                    # normalize: out^T_h = av[0:64] * (1 / av[64]) , bf16
                    for hi in range(2):
                        recip = small.tile([1, 512], f32r, tag="recip",
                                           name=f"rc{hp}_{qb}_{hi}")
                        with nc.allow_low_precision(
                                reason="f32r recip feeds broadcast matmul"):
                            nc.vector.reciprocal(recip[:], av[hi][64:65, :])
                        bc = psum.tile([128, 2, 512], f32, tag="st",
                                       name=f"bc{hp}_{qb}_{hi}")
                        nc.tensor.matmul(bc[0:64, 0, :], ones_sb[:], recip[:],
                                         start=True, stop=True)
                        bcs = small.tile([64, 512], f32, tag="bcs",
                                         name=f"bcs{hp}_{qb}_{hi}")
                        nc.vector.tensor_copy(bcs[:], bc[0:64, 0, :])
                        nc.vector.tensor_mul(
                            outT_sb[hi * 64:(hi + 1) * 64, hp,
                                    qb * 512:(qb + 1) * 512],
                            av[hi][0:64, :], bcs[:])

            # ---- phase 3: output projection ----
            # out[q, F] = sum_hd outT[hd, q]^T x wo[hd, F]
            for qc in range(QSH // 128):
                acc = psum.tile([128, 512], f32, tag="proj")
                for hdc in range(HDC):
                    nc.tensor.matmul(
                        acc[:],
                        outT_sb[:, hdc, qc * 128:(qc + 1) * 128],
                        wo_bf[:, hdc, :],
                        start=(hdc == 0), stop=(hdc == HDC - 1),
                    )
                ostage = small.tile([128, 512], f32, tag="ost", bufs=2)
                nc.vector.tensor_copy(ostage[:], acc[:])
                nc.sync.dma_start(out=out[qc * 128:(qc + 1) * 128, :],
                                  in_=ostage[:])

    nc.compile()
    return nc


def kernel(**inputs):
    global last_result
    inputs_q = np.asarray(inputs["inputs_q"], dtype=np.float32)
    inputs_kv = np.asarray(inputs["inputs_kv"], dtype=np.float32)
    Wq = np.asarray(inputs["Wq"], dtype=np.float32).reshape(F, HD)
    Wk = np.asarray(inputs["Wk"], dtype=np.float32).reshape(F, HD)
    Wv = np.asarray(inputs["Wv"], dtype=np.float32).reshape(F, HD)
    Wo = np.asarray(inputs["Wo"], dtype=np.float32).reshape(HD, F)
    ones = np.ones((1, 64), dtype=np.float32)

    if "nc" not in _cache:
        _cache["nc"] = _build_program()
    nc = _cache["nc"]

    xkvT = [np.ascontiguousarray(inputs_kv[b].T) for b in range(B)]
    in_maps = []
    for c in range(NCORES):
        b, qi = c // 4, c % 4
        in_maps.append({
            "xqT": np.ascontiguousarray(
                inputs_q[b, qi * QSH:(qi + 1) * QSH, :].T),
            "xkvT": xkvT[b],
            "wq": Wq, "wk": Wk, "wv": Wv, "wo": Wo,
            "ones64": ones,
        })

    res = run_bass_kernel_spmd(nc, in_maps, core_ids=list(range(NCORES)))
    last_result = res

    out = np.empty((B, Q, F), dtype=np.float32)
    for c in range(NCORES):
        b, qi = c // 4, c % 4
        out[b, qi * QSH:(qi + 1) * QSH, :] = res.results[c]["out"]
    return out


# revision 4
# speedup vs baseline: 1.1511x; 1.1511x over previous
"""Trainium2 Bass kernel for nn_MultiHeadDotProductAttention_76725295776285.

Full multi-head attention (B=2, Q=K=4096, F=512, H=8, D=64) on 8 NeuronCores.

Sharding: core c handles batch b = c//4 and q-rows [(c%4)*1024, (c%4+1)*1024).
Each core computes all 8 heads for its q-slice (K/V projection for its batch is
recomputed on each of the 4 cores sharing that batch), so the output projection
sums over heads locally and no collective is needed.

Device-side dataflow (per core):
  - activations are fed pre-transposed ([F, seq]) so every matmul has its
    contraction dim on partitions with no on-chip transposes
  - Q^T [hd, q], K^T [hd, k] kept in float32r (TF32-like, full PE rate)
  - V evacuated to bf16 as [k, head, 65] with a ones column, so the AV matmul
    produces the softmax denominator in its 65th output row for free
  - S^T = K_h Q_h^T per head via two row-packed (tile_position) Kc=64 matmuls
  - softmax without max-subtraction (logits std ~8, |logit| < ~60 is fp32-safe)
  - exp on the Scalar engine PSUM->SBUF in [128, 1024] slabs, bf16 out
  - out^T accumulated in PSUM over all 32 k-chunks, normalized by the
    reciprocal denominator broadcast across partitions with a tiny matmul
  - output projection in bf16, accumulate over hd chunks, DMA out fp32
"""

import os
import sys

for _p in ("/opt/trn_rl_repo", "/root/.axon_site/_ro/trn_rl_repo"):
    if os.path.isdir(_p) and _p not in sys.path:
        sys.path.append(_p)

import numpy as np

import concourse.bacc as bacc
import concourse.tile as tile
from concourse import mybir
from concourse.bass_utils import run_bass_kernel_spmd

B, Q, K, F, H, D = 2, 4096, 4096, 512, 8, 64
HD = H * D            # 512
NCORES = 8
QSH = Q // 4          # 1024 q rows per core
FC = F // 128         # 4 F chunks
HDC = HD // 128       # 4 hd chunks
NKB = K // 512        # 8 k blocks (DMA/projection granularity)
NKC = K // 128        # 32 k chunks (attention granularity)
NQB = QSH // 512      # 2 q blocks per core
NHP = H // 2          # 4 head pairs

f32 = mybir.dt.float32
f32r = mybir.dt.float32r
f16 = mybir.dt.float16
bf16 = mybir.dt.bfloat16

_cache = {}
last_result = None  # BassKernelResults of the most recent run (for profiling)


def _build_program():
    nc = bacc.Bacc("TRN2", target_bir_lowering=False, debug=False,
                   num_devices=NCORES)

    xqT = nc.dram_tensor("xqT", [F, QSH], f16, kind="ExternalInput")
    xkvT = nc.dram_tensor("xkvT", [F, K], f16, kind="ExternalInput")
    wq = nc.dram_tensor("wq", [F, HD], f16, kind="ExternalInput")
    wk = nc.dram_tensor("wk", [F, HD], f16, kind="ExternalInput")
    wv = nc.dram_tensor("wv", [F, HD], f16, kind="ExternalInput")
    wo = nc.dram_tensor("wo", [HD, F], bf16, kind="ExternalInput")
    ones64 = nc.dram_tensor("ones64", [1, 64], f32r, kind="ExternalInput")
    out = nc.dram_tensor("out", [QSH, F], f32, kind="ExternalOutput")

    # partition-major views: row index (c*128 + p) -> [p, c, :]
    xqT_r = xqT.rearrange("(c p) q -> p c q", p=128)
    xkvT_r = xkvT.rearrange("(c p) k -> p c k", p=128)
    wq_r = wq.rearrange("(c p) n -> p c n", p=128)
    wk_r = wk.rearrange("(c p) n -> p c n", p=128)
    wv_r = wv.rearrange("(c p) n -> p c n", p=128)
    wo_r = wo.rearrange("(c p) n -> p c n", p=128)

    with tile.TileContext(nc) as tc:
        with (
            tc.tile_pool(name="persist", bufs=1) as persist,
            tc.tile_pool(name="stream", bufs=2) as stream,
            tc.tile_pool(name="ptp", bufs=4) as ptp,
            tc.tile_pool(name="small", bufs=4) as small,
            tc.tile_pool(name="psum", bufs=2, space="PSUM") as psum,
        ):
            # ---- persistent SBUF tensors ----
            qT_sb = persist.tile([128, HDC, QSH], f16, tag="qT")
            kT_sb = [persist.tile([128, HDC, 512], f16, tag=f"kT{kb}",
                                  name=f"kT{kb}")
                     for kb in range(NKB)]
            v_sb = [persist.tile([128, H, 65], bf16, tag=f"v{kc}",
                                 name=f"v{kc}")
                    for kc in range(NKC)]
            outT_sb = persist.tile([128, HDC, QSH], bf16, tag="outT")
            wk_sb = persist.tile([128, FC, HD], f16, tag="wk")
            wv_sb = persist.tile([128, FC, HD], f16, tag="wv")
            wq_sb = persist.tile([128, FC, HD], f16, tag="wq")
            wo_bf = persist.tile([128, HDC, F], bf16, tag="wo_bf")
            ones_sb = persist.tile([1, 64], f32r, tag="ones")

            # ---- input DMAs (HWDGE via sync engine) ----
            nc.sync.dma_start(out=wq_sb[:], in_=wq_r[:])
            nc.sync.dma_start(out=wk_sb[:], in_=wk_r[:])
            nc.sync.dma_start(out=wv_sb[:], in_=wv_r[:])
            nc.sync.dma_start(out=ones_sb[:], in_=ones64[:])
            nc.sync.dma_start(out=wo_bf[:], in_=wo_r[:])

            # ---- phase 1a: Q^T projection ----
            # Q^T[hd, q] = sum_F wq[F, hd]^T x xqT[F, q]
            for qb in range(NQB):
                xq_blk = stream.tile([128, FC, 512], f16, tag="xblk")
                nc.sync.dma_start(out=xq_blk[:],
                                  in_=xqT_r[:, :, qb * 512:(qb + 1) * 512])
                for hdc in range(HDC):
                    acc = psum.tile([128, 512], f32, tag="proj")
                    for fc in range(FC):
                        nc.tensor.matmul(
                            acc[:],
                            wq_sb[:, fc, hdc * 128:(hdc + 1) * 128],
                            xq_blk[:, fc, :],
                            start=(fc == 0), stop=(fc == FC - 1),
                        )
                    nc.vector.tensor_copy(
                        qT_sb[:, hdc, qb * 512:(qb + 1) * 512], acc[:])

            # ---- phase 1b: K^T and V projections, per k block ----
            for kb in range(NKB):
                xkv_blk = stream.tile([128, FC, 512], f16, tag="xblk")
                nc.sync.dma_start(out=xkv_blk[:],
                                  in_=xkvT_r[:, :, kb * 512:(kb + 1) * 512])
                # K^T[hd, kblock]
                for hdc in range(HDC):
                    acc = psum.tile([128, 512], f32, tag="proj")
                    for fc in range(FC):
                        nc.tensor.matmul(
                            acc[:],
                            wk_sb[:, fc, hdc * 128:(hdc + 1) * 128],
                            xkv_blk[:, fc, :],
                            start=(fc == 0), stop=(fc == FC - 1),
                        )
                    nc.vector.tensor_copy(kT_sb[kb][:, hdc, :], acc[:])
                # V[kblock, hd] (natural layout, k on partitions)
                for ks in range(4):
                    kc = kb * 4 + ks
                    acc = psum.tile([128, 512], f32, tag="proj")
                    for fc in range(FC):
                        nc.tensor.matmul(
                            acc[:],
                            xkv_blk[:, fc, ks * 128:(ks + 1) * 128],
                            wv_sb[:, fc, :],
                            start=(fc == 0), stop=(fc == FC - 1),
                        )
                    nc.vector.tensor_copy(
                        v_sb[kc][:, :, 0:64],
                        acc.rearrange("p (h d) -> p h d", h=H))
                    nc.vector.memset(v_sb[kc][:, :, 64:65], 1.0)

            # ---- phase 2: attention, per head pair / q block ----
            for hp in range(NHP):
                for qb in range(NQB):
                    av = [psum.tile([65, 512], f32, tag="av", name=f"av{hp}_{qb}_{hi}")
                          for hi in range(2)]
                    for kc in range(NKC):
                        kb, ks = kc // 4, kc % 4
                        st = psum.tile([128, 2, 512], f32, tag="st",
                                       name=f"st{hp}_{qb}_{kc}")
                        for hi in range(2):
                            nc.tensor.matmul(
                                st[:, hi, :],
                                kT_sb[kb][hi * 64:(hi + 1) * 64, hp,
                                          ks * 128:(ks + 1) * 128],
                                qT_sb[hi * 64:(hi + 1) * 64, hp,
                                      qb * 512:(qb + 1) * 512],
                                start=True, stop=True,
                                tile_position=(hi * 64, 0),
                            )
                        pT = ptp.tile([128, 2, 512], bf16, tag="pT",
                                      name=f"pT{hp}_{qb}_{kc}")
                        nc.scalar.activation(
                            out=pT[:], in_=st[:],
                            func=mybir.ActivationFunctionType.Exp)
                        for hi in range(2):
                            nc.tensor.matmul(
                                av[hi][:],
                                v_sb[kc][:, hp * 2 + hi, :],
                                pT[:, hi, :],
                                start=(kc == 0), stop=(kc == NKC - 1),
                            )
                    # normalize: out^T_h = av[0:64] * (1 / av[64]) , bf16
                    for hi in range(2):
                        recip = small.tile([1, 512], f32r, tag="recip",
                                           name=f"rc{hp}_{qb}_{hi}")
                        with nc.allow_low_precision(
                                reason="f32r recip feeds broadcast matmul"):
                            nc.vector.reciprocal(recip[:], av[hi][64:65, :])
                        bc = psum.tile([128, 2, 512], f32, tag="st",
                                       name=f"bc{hp}_{qb}_{hi}")
                        nc.tensor.matmul(bc[0:64, 0, :], ones_sb[:], recip[:],
                                         start=True, stop=True)
                        bcs = small.tile([64, 512], f32, tag="bcs",
                                         name=f"bcs{hp}_{qb}_{hi}")
                        nc.vector.tensor_copy(bcs[:], bc[0:64, 0, :])
                        nc.vector.tensor_mul(
                            outT_sb[hi * 64:(hi + 1) * 64, hp,
                                    qb * 512:(qb + 1) * 512],
                            av[hi][0:64, :], bcs[:])

            # ---- phase 3: output projection ----
            # out[q, F] = sum_hd outT[hd, q]^T x wo[hd, F]
            for qc in range(QSH // 128):
                acc = psum.tile([128, 512], f32, tag="proj")
                for hdc in range(HDC):
                    nc.tensor.matmul(
                        acc[:],
                        outT_sb[:, hdc, qc * 128:(qc + 1) * 128],
                        wo_bf[:, hdc, :],
                        start=(hdc == 0), stop=(hdc == HDC - 1),
                    )
                ostage = small.tile([128, 512], f32, tag="ost", bufs=2)
                nc.vector.tensor_copy(ostage[:], acc[:])
                nc.sync.dma_start(out=out[qc * 128:(qc + 1) * 128, :],
                                  in_=ostage[:])

    nc.compile()
    return nc


def kernel(**inputs):
    global last_result
    import ml_dtypes
    inputs_q = np.asarray(inputs["inputs_q"], dtype=np.float32)
    inputs_kv = np.asarray(inputs["inputs_kv"], dtype=np.float32)
    Wq = np.asarray(inputs["Wq"], dtype=np.float32).reshape(F, HD).astype(np.float16)
    Wk = np.asarray(inputs["Wk"], dtype=np.float32).reshape(F, HD).astype(np.float16)
    Wv = np.asarray(inputs["Wv"], dtype=np.float32).reshape(F, HD).astype(np.float16)
    Wo = np.asarray(inputs["Wo"], dtype=np.float32).reshape(HD, F).astype(ml_dtypes.bfloat16)
    ones = np.ones((1, 64), dtype=np.float32)

    if "nc" not in _cache:
        _cache["nc"] = _build_program()
    nc = _cache["nc"]

    xkvT = [np.ascontiguousarray(inputs_kv[b].T).astype(np.float16) for b in range(B)]
    in_maps = []
    for c in range(NCORES):
        b, qi = c // 4, c % 4
        in_maps.append({
            "xqT": np.ascontiguousarray(
                inputs_q[b, qi * QSH:(qi + 1) * QSH, :].T).astype(np.float16),
            "xkvT": xkvT[b],
            "wq": Wq, "wk": Wk, "wv": Wv, "wo": Wo,
            "ones64": ones,
        })

    res = run_bass_kernel_spmd(nc, in_maps, core_ids=list(range(NCORES)))
    last_result = res

    out = np.empty((B, Q, F), dtype=np.float32)
    for c in range(NCORES):
        b, qi = c // 4, c % 4
        out[b, qi * QSH:(qi + 1) * QSH, :] = res.results[c]["out"]
    return out


# revision 6
# speedup vs baseline: 1.1927x; 1.0362x over previous
"""Trainium2 Bass kernel for nn_MultiHeadDotProductAttention_76725295776285.

Full multi-head attention (B=2, Q=K=4096, F=512, H=8, D=64) on 8 NeuronCores.

Sharding: core c handles batch b = c//4 and q-rows [(c%4)*1024, (c%4+1)*1024).
Each core computes all 8 heads for its q-slice (K/V projection for its batch is
recomputed on each of the 4 cores sharing that batch), so the output projection
sums over heads locally and no collective is needed.

Device-side dataflow (per core):
  - activations are fed pre-transposed ([F, seq]) so every matmul has its
    contraction dim on partitions with no on-chip transposes
  - Q^T [hd, q], K^T [hd, k] kept in float32r (TF32-like, full PE rate)
  - V evacuated to bf16 as [k, head, 65] with a ones column, so the AV matmul
    produces the softmax denominator in its 65th output row for free
  - S^T = K_h Q_h^T per head via two row-packed (tile_position) Kc=64 matmuls
  - softmax without max-subtraction (logits std ~8, |logit| < ~60 is fp32-safe)
  - exp on the Scalar engine PSUM->SBUF in [128, 1024] slabs, bf16 out
  - out^T accumulated in PSUM over all 32 k-chunks, normalized by the
    reciprocal denominator broadcast across partitions with a tiny matmul
  - output projection in bf16, accumulate over hd chunks, DMA out fp32
"""

import os
import sys

for _p in ("/opt/trn_rl_repo", "/root/.axon_site/_ro/trn_rl_repo"):
    if os.path.isdir(_p) and _p not in sys.path:
        sys.path.append(_p)

import numpy as np

import concourse.bacc as bacc
import concourse.tile as tile
from concourse import mybir
from concourse.bass_utils import run_bass_kernel_spmd

B, Q, K, F, H, D = 2, 4096, 4096, 512, 8, 64
HD = H * D            # 512
NCORES = 8
QSH = Q // 4          # 1024 q rows per core
FC = F // 128         # 4 F chunks
HDC = HD // 128       # 4 hd chunks
NKB = K // 512        # 8 k blocks (DMA/projection granularity)
NKC = K // 128        # 32 k chunks (attention granularity)
NQB = QSH // 512      # 2 q blocks per core
NHP = H // 2          # 4 head pairs

f32 = mybir.dt.float32
f32r = mybir.dt.float32r
f16 = mybir.dt.float16
bf16 = mybir.dt.bfloat16

_cache = {}
last_result = None  # BassKernelResults of the most recent run (for profiling)


def _build_program():
    nc = bacc.Bacc("TRN2", target_bir_lowering=False, debug=False,
                   num_devices=NCORES)

    xqT = nc.dram_tensor("xqT", [F, QSH], f16, kind="ExternalInput")
    xkvT = nc.dram_tensor("xkvT", [F, K], f16, kind="ExternalInput")
    wq = nc.dram_tensor("wq", [F, HD], f16, kind="ExternalInput")
    wk = nc.dram_tensor("wk", [F, HD], f16, kind="ExternalInput")
    wv = nc.dram_tensor("wv", [F, HD], f16, kind="ExternalInput")
    wo = nc.dram_tensor("wo", [HD, F], bf16, kind="ExternalInput")
    ones64 = nc.dram_tensor("ones64", [1, 64], f32r, kind="ExternalInput")
    out = nc.dram_tensor("out", [QSH, F], f32, kind="ExternalOutput")

    # partition-major views: row index (c*128 + p) -> [p, c, :]
    xqT_r = xqT.rearrange("(c p) q -> p c q", p=128)
    xkvT_r = xkvT.rearrange("(c p) k -> p c k", p=128)
    wq_r = wq.rearrange("(c p) n -> p c n", p=128)
    wk_r = wk.rearrange("(c p) n -> p c n", p=128)
    wv_r = wv.rearrange("(c p) n -> p c n", p=128)
    wo_r = wo.rearrange("(c p) n -> p c n", p=128)

    with tile.TileContext(nc) as tc:
        with (
            tc.tile_pool(name="persist", bufs=1) as persist,
            tc.tile_pool(name="stream", bufs=2) as stream,
            tc.tile_pool(name="ptp", bufs=12) as ptp,
            tc.tile_pool(name="small", bufs=4) as small,
            tc.tile_pool(name="psum", bufs=2, space="PSUM") as psum,
        ):
            # ---- persistent SBUF tensors ----
            qT_sb = persist.tile([128, HDC, QSH], f16, tag="qT")
            kT_sb = [persist.tile([128, HDC, 512], f16, tag=f"kT{kb}",
                                  name=f"kT{kb}")
                     for kb in range(NKB)]
            v_sb = [persist.tile([128, H, 65], bf16, tag=f"v{kc}",
                                 name=f"v{kc}")
                    for kc in range(NKC)]
            outT_sb = persist.tile([128, HDC, QSH], bf16, tag="outT")
            wk_sb = persist.tile([128, FC, HD], f16, tag="wk")
            wv_sb = persist.tile([128, FC, HD], f16, tag="wv")
            wq_sb = persist.tile([128, FC, HD], f16, tag="wq")
            wo_bf = persist.tile([128, HDC, F], bf16, tag="wo_bf")
            ones_sb = persist.tile([1, 64], f32r, tag="ones")

            # ---- input DMAs (HWDGE via sync engine) ----
            nc.sync.dma_start(out=wq_sb[:], in_=wq_r[:])
            nc.sync.dma_start(out=wk_sb[:], in_=wk_r[:])
            nc.sync.dma_start(out=wv_sb[:], in_=wv_r[:])
            nc.sync.dma_start(out=ones_sb[:], in_=ones64[:])
            nc.sync.dma_start(out=wo_bf[:], in_=wo_r[:])

            # ---- phase 1a: Q^T projection ----
            # Q^T[hd, q] = sum_F wq[F, hd]^T x xqT[F, q]
            for qb in range(NQB):
                xq_blk = stream.tile([128, FC, 512], f16, tag="xblk")
                nc.sync.dma_start(out=xq_blk[:],
                                  in_=xqT_r[:, :, qb * 512:(qb + 1) * 512])
                for hdc in range(HDC):
                    acc = psum.tile([128, 512], f32, tag="proj")
                    for fc in range(FC):
                        nc.tensor.matmul(
                            acc[:],
                            wq_sb[:, fc, hdc * 128:(hdc + 1) * 128],
                            xq_blk[:, fc, :],
                            start=(fc == 0), stop=(fc == FC - 1),
                        )
                    nc.vector.tensor_copy(
                        qT_sb[:, hdc, qb * 512:(qb + 1) * 512], acc[:])

            # ---- phase 1b: K^T and V projections, per k block ----
            for kb in range(NKB):
                xkv_blk = stream.tile([128, FC, 512], f16, tag="xblk")
                nc.sync.dma_start(out=xkv_blk[:],
                                  in_=xkvT_r[:, :, kb * 512:(kb + 1) * 512])
                # K^T[hd, kblock]
                for hdc in range(HDC):
                    acc = psum.tile([128, 512], f32, tag="proj")
                    for fc in range(FC):
                        nc.tensor.matmul(
                            acc[:],
                            wk_sb[:, fc, hdc * 128:(hdc + 1) * 128],
                            xkv_blk[:, fc, :],
                            start=(fc == 0), stop=(fc == FC - 1),
                        )
                    nc.vector.tensor_copy(kT_sb[kb][:, hdc, :], acc[:])
                # V[kblock, hd] (natural layout, k on partitions)
                for ks in range(4):
                    kc = kb * 4 + ks
                    acc = psum.tile([128, 512], f32, tag="proj")
                    for fc in range(FC):
                        nc.tensor.matmul(
                            acc[:],
                            xkv_blk[:, fc, ks * 128:(ks + 1) * 128],
                            wv_sb[:, fc, :],
                            start=(fc == 0), stop=(fc == FC - 1),
                        )
                    nc.vector.tensor_copy(
                        v_sb[kc][:, :, 0:64],
                        acc.rearrange("p (h d) -> p h d", h=H))
                    nc.vector.memset(v_sb[kc][:, :, 64:65], 1.0)

            # ---- phase 2: attention, per head pair / q block ----
            for hp in range(NHP):
                for qb in range(NQB):
                    av = [psum.tile([65, 512], f32, tag="av", name=f"av{hp}_{qb}_{hi}")
                          for hi in range(2)]
                    for kc in range(NKC):
                        kb, ks = kc // 4, kc % 4
                        st = psum.tile([128, 2, 512], f32, tag="st",
                                       name=f"st{hp}_{qb}_{kc}")
                        for hi in range(2):
                            nc.tensor.matmul(
                                st[:, hi, :],
                                kT_sb[kb][hi * 64:(hi + 1) * 64, hp,
                                          ks * 128:(ks + 1) * 128],
                                qT_sb[hi * 64:(hi + 1) * 64, hp,
                                      qb * 512:(qb + 1) * 512],
                                start=True, stop=True,
                                tile_position=(hi * 64, 0),
                            )
                        pT = ptp.tile([128, 2, 512], bf16, tag="pT",
                                      name=f"pT{hp}_{qb}_{kc}")
                        nc.scalar.activation(
                            out=pT[:], in_=st[:],
                            func=mybir.ActivationFunctionType.Exp)
                        for hi in range(2):
                            nc.tensor.matmul(
                                av[hi][:],
                                v_sb[kc][:, hp * 2 + hi, :],
                                pT[:, hi, :],
                                start=(kc == 0), stop=(kc == NKC - 1),
                            )
                    # normalize: out^T_h = av[0:64] * (1 / av[64]) , bf16.
                    # First evacuate av to SBUF (frees the PSUM slot fast so
                    # the next head-pair's AV accumulation isn't stalled by
                    # the slow reciprocal), then normalize off-critical-path.
                    for hi in range(2):
                        u = small.tile([65, 512], f32, tag="uav",
                                       name=f"u{hp}_{qb}_{hi}")
                        nc.vector.tensor_copy(u[:], av[hi][:])
                        recip = small.tile([1, 512], f32r, tag="recip",
                                           name=f"rc{hp}_{qb}_{hi}")
                        with nc.allow_low_precision(
                                reason="f32r recip feeds broadcast matmul"):
                            nc.vector.reciprocal(recip[:], u[64:65, :])
                        bc = psum.tile([65, 512], f32, tag="av",
                                       name=f"bc{hp}_{qb}_{hi}")
                        nc.tensor.matmul(bc[0:64, :], ones_sb[:], recip[:],
                                         start=True, stop=True)
                        nc.vector.tensor_mul(
                            outT_sb[hi * 64:(hi + 1) * 64, hp,
                                    qb * 512:(qb + 1) * 512],
                            u[0:64, :], bc[0:64, :])

            # ---- phase 3: output projection ----
            # out[q, F] = sum_hd outT[hd, q]^T x wo[hd, F]
            for qc in range(QSH // 128):
                acc = psum.tile([128, 512], f32, tag="proj")
                for hdc in range(HDC):
                    nc.tensor.matmul(
                        acc[:],
                        outT_sb[:, hdc, qc * 128:(qc + 1) * 128],
                        wo_bf[:, hdc, :],
                        start=(hdc == 0), stop=(hdc == HDC - 1),
                    )
                ostage = small.tile([128, 512], f32, tag="ost", bufs=2)
                nc.vector.tensor_copy(ostage[:], acc[:])
                nc.sync.dma_start(out=out[qc * 128:(qc + 1) * 128, :],
                                  in_=ostage[:])

    nc.compile()
    return nc


def kernel(**inputs):
    global last_result
    import ml_dtypes
    inputs_q = np.asarray(inputs["inputs_q"], dtype=np.float32)
    inputs_kv = np.asarray(inputs["inputs_kv"], dtype=np.float32)
    Wq = np.asarray(inputs["Wq"], dtype=np.float32).reshape(F, HD).astype(np.float16)
    Wk = np.asarray(inputs["Wk"], dtype=np.float32).reshape(F, HD).astype(np.float16)
    Wv = np.asarray(inputs["Wv"], dtype=np.float32).reshape(F, HD).astype(np.float16)
    Wo = np.asarray(inputs["Wo"], dtype=np.float32).reshape(HD, F).astype(ml_dtypes.bfloat16)
    ones = np.ones((1, 64), dtype=np.float32)

    if "nc" not in _cache:
        _cache["nc"] = _build_program()
    nc = _cache["nc"]

    xkvT = [np.ascontiguousarray(inputs_kv[b].T).astype(np.float16) for b in range(B)]
    in_maps = []
    for c in range(NCORES):
        b, qi = c // 4, c % 4
        in_maps.append({
            "xqT": np.ascontiguousarray(
                inputs_q[b, qi * QSH:(qi + 1) * QSH, :].T).astype(np.float16),
            "xkvT": xkvT[b],
            "wq": Wq, "wk": Wk, "wv": Wv, "wo": Wo,
            "ones64": ones,
        })

    res = run_bass_kernel_spmd(nc, in_maps, core_ids=list(range(NCORES)))
    last_result = res

    out = np.empty((B, Q, F), dtype=np.float32)
    for c in range(NCORES):
        b, qi = c // 4, c % 4
        out[b, qi * QSH:(qi + 1) * QSH, :] = res.results[c]["out"]
    return out
